# revision 17
# baseline (speedup 1.0000x reference)
"""Trainium2 Bass kernel for nn_DEC_21732534517794.

Two independent GRU+attention decoders over B=128, L=100, H=128.
Sharding: 8 cores = 2 decoders x 4 batch-shards (B_local=32 per core).
Layout: [H=128 partitions, 32 batch free], attention slots a-BLOCKED
(col = a*32 + b for the two attends a=0: query h0n, a=1: query h1n).

v2 structure (vs v1):
  - keys cache keysN[t, b*128+h] (non-parity, t on partitions).  Only h0n
    keys are cached; each step's new keys (h0n_i, h1n_i) contribute to the
    context via a PE row-broadcast of their softmax weights + DVE muls, so
    the cache append (a 32-descriptor partition-gather DMA) is consumed one
    step later and its latency hides.  The DMA is split Pool/SP halves.
  - softmax normalized in Ea space: exp with accum_out gives the denom for
    free, reciprocal + per-partition tensor_scalar scale, then DVE square
    transposes produce E2n[t, a*32+b]; weighted sum = 32 matmuls
    lhsT=keysN[0:i, b-block] rhs=E2n[0:i, {b, 32+b}].
  - e = tanh(kW + sW) with the rep-8 broadcast add (2x bf16 DVE), split in
    two chunks so the second add overlaps the first tanh.
  - GRU gates as in v1 (PE matmuls + tanh-folded sigmoid), fc in bf16,
    state update via DVE tensor_scalar (no ACT).
"""

from contextlib import ExitStack

import numpy as np
import ml_dtypes

import concourse.bass as bass
import concourse.bacc as bacc
import concourse.mybir as mybir
import concourse.tile as tile
from concourse.bass_utils import run_bass_kernel_spmd
from concourse.dve_ops import AFFINE_MUL_REDUCE

F32 = mybir.dt.float32
BF16 = mybir.dt.bfloat16
AF = mybir.ActivationFunctionType
Alu = mybir.AluOpType
H = 128
BC = 32          # batch per core
TAL = 104        # per-b t-columns allocated in kW / e caches
NSTEPS = 100


def _ap(t, offset, dims):
    """AP on tile t with explicit free dims (partition dim kept full)."""
    base = t[:]
    return bass.AP(tensor=base.tensor, offset=base.offset + offset,
                   ap=[base.ap[0]] + [list(d) for d in dims])


def _app(t, p0, pn, offset, dims):
    """AP with partition range [p0, p0+pn) and explicit free dims."""
    base = t[p0:p0 + pn, :]
    return bass.AP(tensor=base.tensor, offset=base.offset + offset,
                   ap=[base.ap[0]] + [list(d) for d in dims])


def build(nsteps=NSTEPS):
    nc = bacc.Bacc(trn_type="TRN2")
    L = nsteps

    din = {}
    def dt_in(name, shape, dtype=F32):
        din[name] = nc.dram_tensor(name, shape, dtype, kind="ExternalInput")
        return din[name]

    xT = dt_in("xT", [3, L * BC])
    Wih0T = dt_in("Wih0T", [3, 384])
    Whh0T = dt_in("Whh0T", [128, 384])
    Wih1T = dt_in("Wih1T", [128, 384])
    Whh1T = dt_in("Whh1T", [128, 384])
    brz = dt_in("brz", [128, 4])          # cols: r0,z0,r1,z1 (0.5-scaled)
    bin01 = dt_in("bin01", [128, 2])
    bhn_rows = dt_in("bhn_rows", [1, 256])
    WaST = dt_in("WaST", [128, 128], BF16)
    WaKT = dt_in("WaKT", [128, 128], BF16)
    Wv2 = dt_in("Wv2", [128, 1], BF16)
    WfcCT = dt_in("WfcCT", [128, 128], BF16)
    WfcHT = dt_in("WfcHT", [128, 128], BF16)
    bfc1 = dt_in("bfc1", [128, 1])
    WoutX = dt_in("WoutX", [128, 1], BF16)
    I128f = dt_in("I128f", [128, 128])
    ones_r = dt_in("ones_r", [1, BC])
    ones1b = dt_in("ones1b", [1, 128], BF16)
    I128b = dt_in("I128b", [128, 128], BF16)
    sel0 = dt_in("sel0", [2, 128], BF16)
    sel1 = dt_in("sel1", [2, 128], BF16)
    bias2_0 = dt_in("bias2_0", [2, 128])
    bias2_1 = dt_in("bias2_1", [2, 128])
    ind2 = dt_in("ind2", [2, 64])
    mskv = dt_in("mskv", [128, 1])   # -1e30 * sign(Wv)

    p_out = nc.dram_tensor("p_out", [1, L * BC], F32, kind="ExternalOutput")

    with tile.TileContext(nc) as tc, ExitStack() as ctx:
        consts = ctx.enter_context(tc.tile_pool(name="consts", bufs=1))
        state = ctx.enter_context(tc.tile_pool(name="state", bufs=1))
        sp2 = ctx.enter_context(tc.tile_pool(name="sp2", bufs=2))
        sp3 = ctx.enter_context(tc.tile_pool(name="sp3", bufs=3))
        pg = ctx.enter_context(tc.tile_pool(name="pg", bufs=2, space="PSUM"))
        pat = ctx.enter_context(tc.tile_pool(name="pat", bufs=1, space="PSUM"))
        pstate = ctx.enter_context(tc.tile_pool(name="pstate", bufs=1, space="PSUM"))
        plate = ctx.enter_context(tc.tile_pool(name="plate", bufs=1, space="PSUM"))

        # ---- load constants ----
        cs = {}
        for name, t in din.items():
            sh = list(t.shape)
            tl = consts.tile(sh, t.dtype, tag=name)
            nc.gpsimd.dma_start(out=tl[:, :], in_=t[:, :])
            cs[name] = tl

        # ---- persistent state ----
        # hh: h0 at cols 0:32, h1 at 32:64 (true scale); hh_bfc: bf16 copy.
        # hhn2: 2*h0n at 0:32, 2*h1n at 32:64 (per-step, sp2).
        # kW2/e caches: col t*64 + a*32 + b.
        # keysN: row t = h0n key of step t, col b*128 + h (bf16).
        hh = state.tile([128, 64], F32, tag="hh")
        kW2 = state.tile([128, 64 * TAL], BF16, tag="kW2")
        keysN = state.tile([128, BC * 128], BF16, tag="keysN")
        e_in = state.tile([128, 64 * TAL], BF16, tag="e_in")
        e_bf = state.tile([128, 64 * TAL], BF16, tag="e_bf")
        o_all = state.tile([128, L * BC], BF16, tag="o_all")
        Ea = state.tile([64, 128], BF16, tag="Ea")
        Ean = state.tile([64, 128], BF16, tag="Ean")
        E2n = state.tile([128, 64], BF16, tag="E2n")
        dsum = state.tile([64, 1], F32, tag="dsum")
        rdc = state.tile([64, 1], F32, tag="rdc")
        nc.vector.memset(hh[:, :], 0.0)
        nc.vector.memset(kW2[:, :], 0.0)
        nc.vector.memset(e_in[:, :], 0.0)
        nc.vector.memset(e_bf[:, :], 0.0)
        nc.vector.memset(Ea[:, :], 0.0)
        nc.vector.memset(Ean[:, :], 0.0)
        nc.vector.memset(E2n[:, :], 0.0)
        nc.vector.memset(dsum[:, :], 1.0)
        nc.vector.memset(keysN[:, :], 0.0)

        pS = pstate.tile([64, 128], F32, tag="pS")

        MM = nc.tensor.matmul
        ACTV = nc.scalar.activation

        def affine_mul(out, in0, in1, s0, s1):
            nc.vector._custom_dve(AFFINE_MUL_REDUCE, out=out, in0=in0, in1=in1,
                                  s0=float(s0), s1=float(s1))

        def gru_layer(lyr, xg_lhsT, xg_rhs, h_ap, out_ap):
            """One GRU layer. out_ap gets h''=2*h_new."""
            WhhT = cs["Whh0T"] if lyr == 0 else cs["Whh1T"]
            b2 = cs["bias2_0"] if lyr == 0 else cs["bias2_1"]
            prz = pg.tile([128, 64], F32, tag="g")
            pnn = pg.tile([128, 64], F32, tag="g")   # hn | gn
            MM(prz[:, 0:32], xg_lhsT[:, 0:128], xg_rhs, start=True, stop=False)
            MM(prz[:, 0:32], WhhT[:, 0:128], h_ap, start=False, stop=False)
            MM(prz[:, 32:64], xg_lhsT[:, 128:256], xg_rhs, start=True, stop=False)
            MM(prz[:, 32:64], WhhT[:, 128:256], h_ap, start=False, stop=False)
            MM(prz[:, 0:64], b2[0:2, :], cs["ind2"][0:2, :],
               start=False, stop=True, skip_group_check=True)
            MM(pnn[:, 0:32], WhhT[:, 256:384], h_ap, start=True, stop=False)
            MM(pnn[:, 0:32], cs["bhn_rows"][0:1, lyr * 128:(lyr + 1) * 128],
               cs["ones_r"][0:1, :], start=False, stop=True)
            MM(pnn[:, 32:64], xg_lhsT[:, 256:384], xg_rhs, start=True, stop=False)
            trz = sp3.tile([128, 64], F32, tag="trz")
            ACTV(trz[:, 0:64], prz[:, 0:64], AF.Tanh, scale=0.5)
            rhn = sp3.tile([128, 32], F32, tag="rhn")
            nc.vector.scalar_tensor_tensor(
                out=rhn[:, :], in0=trz[:, 0:32], scalar=1.0,
                in1=pnn[:, 0:32], op0=Alu.add, op1=Alu.mult)
            MM(pnn[:, 32:64], cs["I128f"][:, :], rhn[:, :], start=False, stop=True)
            n = sp3.tile([128, 32], F32, tag="n")
            ACTV(n[:, :], pnn[:, 32:64], AF.Tanh,
                 bias=cs["bin01"][:, lyr:lyr + 1], scale=1.0)
            v1 = sp3.tile([128, 32], F32, tag="v1")
            v2 = sp3.tile([128, 32], F32, tag="v2")
            affine_mul(v1[:, :], trz[:, 32:64], h_ap, 1.0, 1.0)     # (1+tz)*h
            affine_mul(v2[:, :], trz[:, 32:64], n[:, :], -1.0, 1.0) # (1-tz)*n
            nc.vector.tensor_add(out_ap, v1[:, :], v2[:, :])

        for i in range(nsteps):
            T = i + 2
            tpad = min(((T + 7) // 8) * 8, TAL)
            ng = tpad // 8

            hhn2 = sp2.tile([128, 64], F32, tag="hhn2")
            gru_layer(0, cs["Wih0T"][0:3, :], cs["xT"][0:3, i * BC:(i + 1) * BC],
                      hh[:, 0:32], hhn2[:, 0:32])
            gru_layer(1, cs["Wih1T"][:, :], hhn2[:, 0:32],
                      hh[:, 32:64], hhn2[:, 32:64])

            hhn2_bf = sp2.tile([128, 64], BF16, tag="hhn2_bf")
            nc.vector.tensor_copy(hhn2_bf[:, :], hhn2[:, :])

            # sW | kW matmuls; rhs = hhn2_bf (a-blocked)
            patt = pat.tile([128, 128], F32, tag="patt")
            MM(patt[:, 0:64], cs["WaST"][:, :], hhn2_bf[:, :], start=True, stop=True)
            MM(patt[:, 64:128], cs["WaKT"][:, :], hhn2_bf[:, :], start=True, stop=True)
            sW_bf = sp2.tile([128, 64], BF16, tag="sW_bf")
            nc.vector.tensor_copy(sW_bf[:, :], patt[:, 0:64])
            # kW2 append: h0n-kW (psum cols 64:96) -> t=i both a; h1n-kW -> t=i+1
            for q, t in ((0, i), (1, i + 1)):
                nc.vector.tensor_copy(
                    _ap(kW2, t * 64, [[32, 2], [1, 32]]),
                    _ap(patt, 64 + 32 * q, [[0, 2], [1, 32]]))

            # e_in = kW + sW (broadcast over t) via rep-8 (2x bf16 DVE),
            # two chunks so chunk-B add overlaps chunk-A tanh.
            nch4 = min(4, ng)
            bnds = [round(c * ng / nch4) for c in range(nch4 + 1)]
            for glo, ghi in zip(bnds[:-1], bnds[1:]):
                gn = ghi - glo
                if gn <= 0:
                    continue
                nc.vector.tensor_add(
                    _ap(e_in, glo * 512, [[64, gn * 8], [1, 64]]),
                    _ap(kW2, glo * 512, [[64, gn * 8], [1, 64]]),
                    _ap(sW_bf, 0, [[0, gn * 8], [1, 64]]))
                # tanh chunk over the valid part of this group range
                lo = glo * 8
                hi = min(T, (glo + gn) * 8)
                if hi > lo:
                    ACTV(e_bf[:, lo * 64:hi * 64], e_in[:, lo * 64:hi * 64], AF.Tanh)
                    # masks belonging to this chunk
                    if lo <= i + 1 < hi:
                        nc.vector.tensor_scalar(
                            _ap(e_bf, (i + 1) * 64, [[1, 32]]),
                            _ap(e_bf, (i + 1) * 64, [[1, 32]]),
                            0.0, cs["mskv"][:, 0:1], Alu.mult, Alu.add)
                    if i == 0 and lo == 0:
                        nc.vector.tensor_scalar(
                            _ap(e_bf, 32, [[1, 32]]),
                            _ap(e_bf, 32, [[1, 32]]),
                            0.0, cs["mskv"][:, 0:1], Alu.mult, Alu.add)
                    # scores for this chunk: out pS[a*32+b, t]
                    for t in range(lo, hi):
                        MM(pS[0:64, t:t + 1], e_bf[:, t * 64:(t + 1) * 64],
                           cs["Wv2"][:, 0:1], start=True, stop=True)

            # keys transpose (for the cache append DMA, consumed next step);
            # emitted here so it fills the DVE gap under the tanh.
            trb = sp2.tile([64, 128], BF16, tag="trb")
            nc.vector.transpose(trb[0:64, 0:64], hhn2_bf[0:64, 0:64])
            nc.vector.transpose(trb[0:64, 64:128], hhn2_bf[64:128, 0:64])
            tb = trb[:, :]
            for half, eng in ((0, nc.gpsimd), (1, nc.sync)):
                src = bass.AP(tensor=tb.tensor,
                              offset=tb.offset + half * 16 * 128,
                              ap=[[128, 16], [1, 128]])
                eng.dma_start(
                    out=keysN[i:i + 1,
                              half * 2048:half * 2048 + 2048].rearrange(
                                  "j (b h) -> j b h", h=128),
                    in_=src)
            nc.gpsimd.tensor_copy(o_all[:, i * BC:(i + 1) * BC],
                                  hhn2_bf[:, 32:64])

            # exp + denominator accumulate, normalize, transpose to E2n[t, ba]
            ACTV(Ea[0:64, 0:T], pS[0:64, 0:T], AF.Exp,
                 accum_out=dsum[0:64, 0:1])
            nc.vector.reciprocal_approx_fast(out=rdc[0:64, 0:1],
                                             in_=dsum[0:64, 0:1])
            nc.vector.tensor_scalar(Ean[0:64, 0:T], Ea[0:64, 0:T],
                                    rdc[0:64, 0:1], None, Alu.mult)
            nc.vector.transpose(E2n[0:64, 0:64], Ean[0:64, 0:64])
            if T > 64:
                nc.vector.transpose(E2n[64:128, 0:64], Ean[0:64, 64:128])

            # weighted sum over cached keys (rows 0..i-1)
            att = plate.tile([128, 160], F32, tag="att")
            pcn = att
            if i >= 1:
                for b in range(BC):
                    MM(_ap(pcn, b, [[32, 2]]),
                       keysN[0:i, b * 128:(b + 1) * 128],
                       _app(E2n, 0, i, b, [[32, 2]]),
                       start=True, stop=True)

            # new-key contributions: transpose Ean cols i, i+1 to a [2, 64]
            # row tile, then PE row-broadcasts via selector constants.
            pEx = plate.tile([2, 64], BF16, tag="pEx")
            nc.tensor.transpose(pEx[0:2, 0:64], Ean[0:64, i:i + 2],
                                cs["I128b"][0:64, 0:64])
            Enew = sp2.tile([2, 64], BF16, tag="Enew")
            nc.vector.tensor_copy(Enew[:, :], pEx[0:2, 0:64])
            MM(att[:, 64:128], cs["sel0"][0:2, :], Enew[0:2, 0:64],
               start=True, stop=True)
            MM(att[:, 128:160], cs["sel1"][0:2, :], Enew[0:2, 32:64],
               start=True, stop=True)
            t0 = sp3.tile([128, 64], F32, tag="t0")
            t1 = sp3.tile([128, 32], F32, tag="t1")
            nc.vector.tensor_tensor(t0[:, :],
                                    _ap(hhn2_bf, 0, [[0, 2], [1, 32]]),
                                    att[:, 64:128], Alu.mult)
            nc.vector.tensor_tensor(t1[:, :], hhn2_bf[:, 32:64],
                                    att[:, 128:160], Alu.mult)
            cT = sp2.tile([128, 64], BF16, tag="cT")
            if i >= 1:
                nc.vector.tensor_add(cT[:, :], pcn[:, 0:64], t0[:, :])
            else:
                nc.vector.tensor_copy(cT[:, :], t0[:, :])
            nc.vector.tensor_add(cT[:, 32:64], cT[:, 32:64], t1[:, :])

            # fc + state update
            pfc = plate.tile([128, 64], F32, tag="pfc")
            MM(pfc[:, :], cs["WfcCT"][:, :], cT[:, :], start=True, stop=False)
            MM(pfc[:, :], cs["WfcHT"][:, :], hhn2_bf[:, :], start=False, stop=True)
            nc.vector.tensor_scalar(hh[:, 0:64], pfc[:, 0:64],
                                    cs["bfc1"][:, 0:1], None, Alu.add)

        # ---- output projection: p[col] for col = t*BC+b, 128 cols per matmul
        NP = L * BC
        assert NP % 128 == 0
        nch = NP // 128
        pp = pstate.tile([128, nch], F32, tag="pp")
        for c in range(nch):
            MM(pp[:, c:c + 1], o_all[:, c * 128:(c + 1) * 128],
               cs["WoutX"][:, 0:1], start=True, stop=True)
        p_sb = sp2.tile([128, nch], F32, tag="p_sb")
        nc.vector.tensor_copy(p_sb[:, :], pp[:, :])
        nc.gpsimd.dma_start(
            out=bass.AP(tensor=p_out[:, :].tensor, offset=0,
                        ap=[[0, 1], [1, 128], [128, nch]]),
            in_=p_sb[:, :])

    nc.compile()
    return nc


def make_inmaps(inputs, nsteps=NSTEPS):
    """Host-side sharding + layout. Returns list of 8 in_maps."""
    f32, bf = np.float32, ml_dtypes.bfloat16
    L = nsteps
    r = {k: np.asarray(v, f32) for k, v in inputs.items()}
    Wfc, Wattn, Wout = r["Wfc"], r["Wattn"], r["Wout"]
    I = np.eye(128, dtype=f32)
    common = {
        "WaST": np.ascontiguousarray((0.5 * Wattn[:, :H]).T).astype(bf),
        "WaKT": np.ascontiguousarray((0.5 * Wattn[:, H:]).T).astype(bf),
        "Wv2": r["Wv"].reshape(128, 1).astype(bf),
        "WfcCT": np.ascontiguousarray((0.5 * Wfc[:, :H]).T).astype(bf),
        "WfcHT": np.ascontiguousarray((0.5 * Wfc[:, H:]).T).astype(bf),
        "bfc1": r["bfc"].reshape(128, 1).astype(f32),
        "I128f": I,
        "ones_r": np.ones((1, BC), f32),
        "ones1b": np.ones((1, 128), bf),
        "I128b": I.astype(bf),
        "sel0": np.stack([np.ones(128, np.float32), np.zeros(128, np.float32)]).astype(bf),
        "sel1": np.stack([np.zeros(128, np.float32), np.ones(128, np.float32)]).astype(bf),
        "mskv": (-1e30 * np.sign(r["Wv"])).reshape(128, 1).astype(f32),
        "ind2": np.concatenate([
            np.concatenate([np.ones((1, 32)), np.zeros((1, 32))], 1),
            np.concatenate([np.zeros((1, 32)), np.ones((1, 32))], 1)]).astype(f32),
    }
    maps = []
    for core in range(8):
        k, shard = core // 4, core % 4
        bsl = slice(shard * BC, (shard + 1) * BC)
        x = r["received"][bsl, :L, :]                       # [32, L, 3]
        xT = np.ascontiguousarray(x.transpose(2, 1, 0)).reshape(3, L * BC)
        brz_cols = np.stack([
            0.5 * (r["bih0"][k][:H] + r["bhh0"][k][:H]),
            0.5 * (r["bih0"][k][H:2 * H] + r["bhh0"][k][H:2 * H]),
            0.5 * (r["bih1"][k][:H] + r["bhh1"][k][:H]),
            0.5 * (r["bih1"][k][H:2 * H] + r["bhh1"][k][H:2 * H])], 1)
        m = dict(common)
        m.update({
            "xT": xT.astype(f32),
            "Wih0T": np.ascontiguousarray(r["Wih0"][k].T).astype(f32),
            "Whh0T": np.ascontiguousarray(
                (r["Whh0"][k] * np.concatenate(
                    [np.ones(2 * H), 0.5 * np.ones(H)])[:, None]).T).astype(f32),
            "Wih1T": np.ascontiguousarray((0.5 * r["Wih1"][k]).T).astype(f32),
            "Whh1T": np.ascontiguousarray(
                (r["Whh1"][k] * np.concatenate(
                    [np.ones(2 * H), 0.5 * np.ones(H)])[:, None]).T).astype(f32),
            "brz": brz_cols.astype(f32),
            "bias2_0": np.stack([r["bih0"][k][:H] + r["bhh0"][k][:H],
                                 r["bih0"][k][H:2 * H] + r["bhh0"][k][H:2 * H]]).astype(f32),
            "bias2_1": np.stack([r["bih1"][k][:H] + r["bhh1"][k][:H],
                                 r["bih1"][k][H:2 * H] + r["bhh1"][k][H:2 * H]]).astype(f32),
            "bin01": np.stack([r["bih0"][k][2 * H:], r["bih1"][k][2 * H:]], 1).astype(f32),
            "bhn_rows": (0.5 * np.concatenate(
                [r["bhh0"][k][2 * H:], r["bhh1"][k][2 * H:]])).reshape(1, 256).astype(f32),
            "WoutX": (0.5 * Wout[0, k * H:(k + 1) * H]).reshape(128, 1).astype(bf),
        })
        maps.append(m)
    return maps


_CACHE = {}


def kernel(**inputs) -> np.ndarray:
    nsteps = NSTEPS
    if "nc" not in _CACHE:
        _CACHE["nc"] = build(nsteps)
    nc = _CACHE["nc"]
    maps = make_inmaps(inputs, nsteps)
    res = run_bass_kernel_spmd(nc, maps, core_ids=list(range(8)))
    outs = res.results
    L = nsteps
    B = 128
    p1 = np.zeros((B, L), np.float32)
    p2 = np.zeros((B, L), np.float32)
    for core in range(8):
        k, shard = core // 4, core % 4
        bsl = slice(shard * BC, (shard + 1) * BC)
        p = np.asarray(outs[core]["p_out"]).reshape(L, BC).T   # [32, L]
        (p1 if k == 0 else p2)[bsl] = p
    bout = float(np.asarray(inputs["bout"]).reshape(-1)[0])
    idx = np.minimum(np.arange(L) + 1, L - 1)
    z = p1 + p2[:, idx] + bout
    out = (1.0 / (1.0 + np.exp(-z))).astype(np.float32)[..., None]
    return out


# revision 18
# speedup vs baseline: 1.1896x; 1.1896x over previous
"""Trainium2 Bass kernel for nn_DEC_21732534517794.

Two independent GRU+attention decoders over B=128, L=100, H=128.
Sharding: 8 cores = 2 decoders x 4 batch-shards (B_local=32 per core).
Layout: [H=128 partitions, 32 batch free], attention slots a-BLOCKED
(col = a*32 + b for the two attends a=0: query h0n, a=1: query h1n).

v2 structure (vs v1):
  - keys cache keysN[t, b*128+h] (non-parity, t on partitions).  Only h0n
    keys are cached; each step's new keys (h0n_i, h1n_i) contribute to the
    context via a PE row-broadcast of their softmax weights + DVE muls, so
    the cache append (a 32-descriptor partition-gather DMA) is consumed one
    step later and its latency hides.  The DMA is split Pool/SP halves.
  - softmax normalized in Ea space: exp with accum_out gives the denom for
    free, reciprocal + per-partition tensor_scalar scale, then DVE square
    transposes produce E2n[t, a*32+b]; weighted sum = 32 matmuls
    lhsT=keysN[0:i, b-block] rhs=E2n[0:i, {b, 32+b}].
  - e = tanh(kW + sW) with a stride-0 sW broadcast add (2x bf16 DVE),
    split in four t-chunks so adds/scores overlap the tanh chain.
  - GRU gates via PE matmuls with r/z biases folded into PSUM (one tanh
    per r|z pair), sigmoid as 0.5+0.5*tanh(0.5x), n-gate hn-path
    pre-halved host-side so rhn = (tr+1)*hn_half in one DVE op; fc in
    bf16, state update via DVE tensor_scalar (no ACT).
"""

from contextlib import ExitStack

import numpy as np
import ml_dtypes

import concourse.bass as bass
import concourse.bacc as bacc
import concourse.mybir as mybir
import concourse.tile as tile
from concourse.bass_utils import run_bass_kernel_spmd
from concourse.dve_ops import AFFINE_MUL_REDUCE

F32 = mybir.dt.float32
BF16 = mybir.dt.bfloat16
AF = mybir.ActivationFunctionType
Alu = mybir.AluOpType
H = 128
BC = 32          # batch per core
TAL = 104        # per-b t-columns allocated in kW / e caches
NSTEPS = 100


def _ap(t, offset, dims):
    """AP on tile t with explicit free dims (partition dim kept full)."""
    base = t[:]
    return bass.AP(tensor=base.tensor, offset=base.offset + offset,
                   ap=[base.ap[0]] + [list(d) for d in dims])


def _app(t, p0, pn, offset, dims):
    """AP with partition range [p0, p0+pn) and explicit free dims."""
    base = t[p0:p0 + pn, :]
    return bass.AP(tensor=base.tensor, offset=base.offset + offset,
                   ap=[base.ap[0]] + [list(d) for d in dims])


def build(nsteps=NSTEPS):
    nc = bacc.Bacc(trn_type="TRN2")
    L = nsteps

    din = {}
    def dt_in(name, shape, dtype=F32):
        din[name] = nc.dram_tensor(name, shape, dtype, kind="ExternalInput")
        return din[name]

    xT = dt_in("xT", [3, L * BC])
    Wih0T = dt_in("Wih0T", [3, 384])
    Whh0T = dt_in("Whh0T", [128, 384])
    Wih1T = dt_in("Wih1T", [128, 384])
    Whh1T = dt_in("Whh1T", [128, 384])
    brz = dt_in("brz", [128, 4])          # cols: r0,z0,r1,z1 (0.5-scaled)
    bin01 = dt_in("bin01", [128, 2])
    bhn_rows = dt_in("bhn_rows", [1, 256])
    WaST = dt_in("WaST", [128, 128], BF16)
    WaKT = dt_in("WaKT", [128, 128], BF16)
    Wv2 = dt_in("Wv2", [128, 1], BF16)
    WfcCT = dt_in("WfcCT", [128, 128], BF16)
    WfcHT = dt_in("WfcHT", [128, 128], BF16)
    bfc1 = dt_in("bfc1", [128, 1])
    WoutX = dt_in("WoutX", [128, 1], BF16)
    I128f = dt_in("I128f", [128, 128])
    ones_r = dt_in("ones_r", [1, BC])
    ones1b = dt_in("ones1b", [1, 128], BF16)
    I128b = dt_in("I128b", [128, 128], BF16)
    sel0 = dt_in("sel0", [2, 128], BF16)
    sel1 = dt_in("sel1", [2, 128], BF16)
    bias2_0 = dt_in("bias2_0", [2, 128])
    bias2_1 = dt_in("bias2_1", [2, 128])
    ind2 = dt_in("ind2", [2, 64])
    mskv = dt_in("mskv", [128, 1])   # -1e30 * sign(Wv)

    p_out = nc.dram_tensor("p_out", [1, L * BC], F32, kind="ExternalOutput")

    with tile.TileContext(nc) as tc, ExitStack() as ctx:
        consts = ctx.enter_context(tc.tile_pool(name="consts", bufs=1))
        state = ctx.enter_context(tc.tile_pool(name="state", bufs=1))
        sp2 = ctx.enter_context(tc.tile_pool(name="sp2", bufs=2))
        sp3 = ctx.enter_context(tc.tile_pool(name="sp3", bufs=3))
        pg = ctx.enter_context(tc.tile_pool(name="pg", bufs=2, space="PSUM"))
        pat = ctx.enter_context(tc.tile_pool(name="pat", bufs=1, space="PSUM"))
        pstate = ctx.enter_context(tc.tile_pool(name="pstate", bufs=1, space="PSUM"))
        plate = ctx.enter_context(tc.tile_pool(name="plate", bufs=1, space="PSUM"))

        # ---- load constants ----
        cs = {}
        for name, t in din.items():
            sh = list(t.shape)
            tl = consts.tile(sh, t.dtype, tag=name)
            nc.gpsimd.dma_start(out=tl[:, :], in_=t[:, :])
            cs[name] = tl

        # ---- persistent state ----
        # hh: h0 at cols 0:32, h1 at 32:64 (true scale); hh_bfc: bf16 copy.
        # hhn2: 2*h0n at 0:32, 2*h1n at 32:64 (per-step, sp2).
        # kW2/e caches: col t*64 + a*32 + b.
        # keysN: row t = h0n key of step t, col b*128 + h (bf16).
        hh = state.tile([128, 64], F32, tag="hh")
        kW2 = state.tile([128, 64 * TAL], BF16, tag="kW2")
        keysN = state.tile([128, BC * 128], BF16, tag="keysN")
        e_in = state.tile([128, 64 * TAL], BF16, tag="e_in")
        e_bf = state.tile([128, 64 * TAL], BF16, tag="e_bf")
        o_all = state.tile([128, L * BC], BF16, tag="o_all")
        Ea = state.tile([64, 128], BF16, tag="Ea")
        Ean = state.tile([64, 128], BF16, tag="Ean")
        E2n = state.tile([128, 64], BF16, tag="E2n")
        dsum = state.tile([64, 1], F32, tag="dsum")
        rdc = state.tile([64, 1], F32, tag="rdc")
        nc.vector.memset(hh[:, :], 0.0)
        nc.vector.memset(kW2[:, :], 0.0)
        nc.vector.memset(e_in[:, :], 0.0)
        nc.vector.memset(e_bf[:, :], 0.0)
        nc.vector.memset(Ea[:, :], 0.0)
        nc.vector.memset(Ean[:, :], 0.0)
        nc.vector.memset(E2n[:, :], 0.0)
        nc.vector.memset(dsum[:, :], 1.0)
        nc.vector.memset(keysN[:, :], 0.0)

        pS = pstate.tile([64, 128], F32, tag="pS")

        MM = nc.tensor.matmul
        ACTV = nc.scalar.activation

        def affine_mul(out, in0, in1, s0, s1):
            nc.vector._custom_dve(AFFINE_MUL_REDUCE, out=out, in0=in0, in1=in1,
                                  s0=float(s0), s1=float(s1))

        def gru_layer(lyr, xg_lhsT, xg_rhs, h_ap, out_ap):
            """One GRU layer. out_ap gets h''=2*h_new."""
            WhhT = cs["Whh0T"] if lyr == 0 else cs["Whh1T"]
            b2 = cs["bias2_0"] if lyr == 0 else cs["bias2_1"]
            prz = pg.tile([128, 64], F32, tag="g")
            pnn = pg.tile([128, 64], F32, tag="g")   # hn | gn
            MM(prz[:, 0:32], xg_lhsT[:, 0:128], xg_rhs, start=True, stop=False)
            MM(prz[:, 0:32], WhhT[:, 0:128], h_ap, start=False, stop=False)
            MM(prz[:, 32:64], xg_lhsT[:, 128:256], xg_rhs, start=True, stop=False)
            MM(prz[:, 32:64], WhhT[:, 128:256], h_ap, start=False, stop=False)
            MM(prz[:, 0:64], b2[0:2, :], cs["ind2"][0:2, :],
               start=False, stop=True, skip_group_check=True)
            MM(pnn[:, 0:32], WhhT[:, 256:384], h_ap, start=True, stop=False)
            MM(pnn[:, 0:32], cs["bhn_rows"][0:1, lyr * 128:(lyr + 1) * 128],
               cs["ones_r"][0:1, :], start=False, stop=True)
            MM(pnn[:, 32:64], xg_lhsT[:, 256:384], xg_rhs, start=True, stop=False)
            trz = sp3.tile([128, 64], F32, tag="trz")
            ACTV(trz[:, 0:64], prz[:, 0:64], AF.Tanh, scale=0.5)
            rhn = sp3.tile([128, 32], F32, tag="rhn")
            nc.vector.scalar_tensor_tensor(
                out=rhn[:, :], in0=trz[:, 0:32], scalar=1.0,
                in1=pnn[:, 0:32], op0=Alu.add, op1=Alu.mult)
            MM(pnn[:, 32:64], cs["I128f"][:, :], rhn[:, :], start=False, stop=True)
            n = sp3.tile([128, 32], F32, tag="n")
            ACTV(n[:, :], pnn[:, 32:64], AF.Tanh,
                 bias=cs["bin01"][:, lyr:lyr + 1], scale=1.0)
            v1 = sp3.tile([128, 32], F32, tag="v1")
            v2 = sp3.tile([128, 32], F32, tag="v2")
            affine_mul(v1[:, :], trz[:, 32:64], h_ap, 1.0, 1.0)     # (1+tz)*h
            affine_mul(v2[:, :], trz[:, 32:64], n[:, :], -1.0, 1.0) # (1-tz)*n
            nc.vector.tensor_add(out_ap, v1[:, :], v2[:, :])

        for i in range(nsteps):
            T = i + 2
            tpad = min(((T + 7) // 8) * 8, TAL)
            ng = tpad // 8

            hhn2 = sp2.tile([128, 64], F32, tag="hhn2")
            gru_layer(0, cs["Wih0T"][0:3, :], cs["xT"][0:3, i * BC:(i + 1) * BC],
                      hh[:, 0:32], hhn2[:, 0:32])
            gru_layer(1, cs["Wih1T"][:, :], hhn2[:, 0:32],
                      hh[:, 32:64], hhn2[:, 32:64])

            hhn2_bf = sp2.tile([128, 64], BF16, tag="hhn2_bf")
            nc.vector.tensor_copy(hhn2_bf[:, :], hhn2[:, :])

            # sW | kW matmuls; rhs = hhn2_bf (a-blocked)
            patt = pat.tile([128, 128], F32, tag="patt")
            MM(patt[:, 0:64], cs["WaST"][:, :], hhn2_bf[:, :], start=True, stop=True)
            MM(patt[:, 64:128], cs["WaKT"][:, :], hhn2_bf[:, :], start=True, stop=True)
            sW_bf = sp2.tile([128, 64], BF16, tag="sW_bf")
            nc.vector.tensor_copy(sW_bf[:, :], patt[:, 0:64])
            # kW2 append: h0n-kW (psum cols 64:96) -> t=i both a; h1n-kW -> t=i+1
            for q, t in ((0, i), (1, i + 1)):
                nc.vector.tensor_copy(
                    _ap(kW2, t * 64, [[32, 2], [1, 32]]),
                    _ap(patt, 64 + 32 * q, [[0, 2], [1, 32]]))

            # e_in = kW + sW (broadcast over t) via rep-8 (2x bf16 DVE),
            # two chunks so chunk-B add overlaps chunk-A tanh.
            nch4 = min(4, ng)
            bnds = [round(c * ng / nch4) for c in range(nch4 + 1)]
            for glo, ghi in zip(bnds[:-1], bnds[1:]):
                gn = ghi - glo
                if gn <= 0:
                    continue
                nc.vector.tensor_add(
                    _ap(e_in, glo * 512, [[64, gn * 8], [1, 64]]),
                    _ap(kW2, glo * 512, [[64, gn * 8], [1, 64]]),
                    _ap(sW_bf, 0, [[0, gn * 8], [1, 64]]))
                # tanh chunk over the valid part of this group range
                lo = glo * 8
                hi = min(T, (glo + gn) * 8)
                if hi > lo:
                    ACTV(e_bf[:, lo * 64:hi * 64], e_in[:, lo * 64:hi * 64], AF.Tanh)
                    # masks belonging to this chunk
                    if lo <= i + 1 < hi:
                        nc.vector.tensor_scalar(
                            _ap(e_bf, (i + 1) * 64, [[1, 32]]),
                            _ap(e_bf, (i + 1) * 64, [[1, 32]]),
                            0.0, cs["mskv"][:, 0:1], Alu.mult, Alu.add)
                    if i == 0 and lo == 0:
                        nc.vector.tensor_scalar(
                            _ap(e_bf, 32, [[1, 32]]),
                            _ap(e_bf, 32, [[1, 32]]),
                            0.0, cs["mskv"][:, 0:1], Alu.mult, Alu.add)
                    # scores for this chunk: out pS[a*32+b, t]
                    for t in range(lo, hi):
                        MM(pS[0:64, t:t + 1], e_bf[:, t * 64:(t + 1) * 64],
                           cs["Wv2"][:, 0:1], start=True, stop=True)

            # keys transpose (for the cache append DMA, consumed next step);
            # emitted here so it fills the DVE gap under the tanh.
            trb = sp2.tile([64, 128], BF16, tag="trb")
            nc.vector.transpose(trb[0:64, 0:64], hhn2_bf[0:64, 0:64])
            nc.vector.transpose(trb[0:64, 64:128], hhn2_bf[64:128, 0:64])
            tb = trb[:, :]
            for half, eng in ((0, nc.gpsimd), (1, nc.sync)):
                src = bass.AP(tensor=tb.tensor,
                              offset=tb.offset + half * 16 * 128,
                              ap=[[128, 16], [1, 128]])
                eng.dma_start(
                    out=keysN[i:i + 1,
                              half * 2048:half * 2048 + 2048].rearrange(
                                  "j (b h) -> j b h", h=128),
                    in_=src)
            nc.gpsimd.tensor_copy(o_all[:, i * BC:(i + 1) * BC],
                                  hhn2_bf[:, 32:64])

            # exp + denominator accumulate, normalize, transpose to E2n[t, ba]
            ACTV(Ea[0:64, 0:T], pS[0:64, 0:T], AF.Exp,
                 accum_out=dsum[0:64, 0:1])
            nc.vector.reciprocal_approx_fast(out=rdc[0:64, 0:1],
                                             in_=dsum[0:64, 0:1])
            nc.vector.tensor_scalar(Ean[0:64, 0:T], Ea[0:64, 0:T],
                                    rdc[0:64, 0:1], None, Alu.mult)
            nc.vector.transpose(E2n[0:64, 0:64], Ean[0:64, 0:64])
            if T > 64:
                nc.vector.transpose(E2n[64:128, 0:64], Ean[0:64, 64:128])

            # weighted sum over cached keys (rows 0..i-1)
            att = plate.tile([128, 160], F32, tag="att")
            pcn = att
            if i >= 1:
                for b in range(BC):
                    MM(_ap(pcn, b, [[32, 2]]),
                       keysN[0:i, b * 128:(b + 1) * 128],
                       _app(E2n, 0, i, b, [[32, 2]]),
                       start=True, stop=True)

            # new-key contributions: transpose Ean cols i, i+1 to a [2, 64]
            # row tile, then PE row-broadcasts via selector constants.
            pEx = plate.tile([2, 64], BF16, tag="pEx")
            nc.tensor.transpose(pEx[0:2, 0:64], Ean[0:64, i:i + 2],
                                cs["I128b"][0:64, 0:64])
            Enew = sp2.tile([2, 64], BF16, tag="Enew")
            nc.vector.tensor_copy(Enew[:, :], pEx[0:2, 0:64])
            MM(att[:, 64:128], cs["sel0"][0:2, :], Enew[0:2, 0:64],
               start=True, stop=True)
            MM(att[:, 128:160], cs["sel1"][0:2, :], Enew[0:2, 32:64],
               start=True, stop=True)
            t0 = sp3.tile([128, 64], F32, tag="t0")
            t1 = sp3.tile([128, 32], F32, tag="t1")
            nc.vector.tensor_tensor(t0[:, :],
                                    _ap(hhn2_bf, 0, [[0, 2], [1, 32]]),
                                    att[:, 64:128], Alu.mult)
            nc.vector.tensor_tensor(t1[:, :], hhn2_bf[:, 32:64],
                                    att[:, 128:160], Alu.mult)
            cT = sp2.tile([128, 64], BF16, tag="cT")
            if i >= 1:
                nc.vector.tensor_add(cT[:, :], pcn[:, 0:64], t0[:, :])
            else:
                nc.vector.tensor_copy(cT[:, :], t0[:, :])
            nc.vector.tensor_add(cT[:, 32:64], cT[:, 32:64], t1[:, :])

            # fc + state update
            pfc = plate.tile([128, 64], F32, tag="pfc")
            MM(pfc[:, :], cs["WfcCT"][:, :], cT[:, :], start=True, stop=False)
            MM(pfc[:, :], cs["WfcHT"][:, :], hhn2_bf[:, :], start=False, stop=True)
            nc.vector.tensor_scalar(hh[:, 0:64], pfc[:, 0:64],
                                    cs["bfc1"][:, 0:1], None, Alu.add)

        # ---- output projection: p[col] for col = t*BC+b, 128 cols per matmul
        NP = L * BC
        assert NP % 128 == 0
        nch = NP // 128
        pp = pstate.tile([128, nch], F32, tag="pp")
        for c in range(nch):
            MM(pp[:, c:c + 1], o_all[:, c * 128:(c + 1) * 128],
               cs["WoutX"][:, 0:1], start=True, stop=True)
        p_sb = sp2.tile([128, nch], F32, tag="p_sb")
        nc.vector.tensor_copy(p_sb[:, :], pp[:, :])
        nc.gpsimd.dma_start(
            out=bass.AP(tensor=p_out[:, :].tensor, offset=0,
                        ap=[[0, 1], [1, 128], [128, nch]]),
            in_=p_sb[:, :])

    nc.compile()
    return nc


def make_inmaps(inputs, nsteps=NSTEPS):
    """Host-side sharding + layout. Returns list of 8 in_maps."""
    f32, bf = np.float32, ml_dtypes.bfloat16
    L = nsteps
    r = {k: np.asarray(v, f32) for k, v in inputs.items()}
    Wfc, Wattn, Wout = r["Wfc"], r["Wattn"], r["Wout"]
    I = np.eye(128, dtype=f32)
    common = {
        "WaST": np.ascontiguousarray((0.5 * Wattn[:, :H]).T).astype(bf),
        "WaKT": np.ascontiguousarray((0.5 * Wattn[:, H:]).T).astype(bf),
        "Wv2": r["Wv"].reshape(128, 1).astype(bf),
        "WfcCT": np.ascontiguousarray((0.5 * Wfc[:, :H]).T).astype(bf),
        "WfcHT": np.ascontiguousarray((0.5 * Wfc[:, H:]).T).astype(bf),
        "bfc1": r["bfc"].reshape(128, 1).astype(f32),
        "I128f": I,
        "ones_r": np.ones((1, BC), f32),
        "ones1b": np.ones((1, 128), bf),
        "I128b": I.astype(bf),
        "sel0": np.stack([np.ones(128, np.float32), np.zeros(128, np.float32)]).astype(bf),
        "sel1": np.stack([np.zeros(128, np.float32), np.ones(128, np.float32)]).astype(bf),
        "mskv": (-1e30 * np.sign(r["Wv"])).reshape(128, 1).astype(f32),
        "ind2": np.concatenate([
            np.concatenate([np.ones((1, 32)), np.zeros((1, 32))], 1),
            np.concatenate([np.zeros((1, 32)), np.ones((1, 32))], 1)]).astype(f32),
    }
    maps = []
    for core in range(8):
        k, shard = core // 4, core % 4
        bsl = slice(shard * BC, (shard + 1) * BC)
        x = r["received"][bsl, :L, :]                       # [32, L, 3]
        xT = np.ascontiguousarray(x.transpose(2, 1, 0)).reshape(3, L * BC)
        brz_cols = np.stack([
            0.5 * (r["bih0"][k][:H] + r["bhh0"][k][:H]),
            0.5 * (r["bih0"][k][H:2 * H] + r["bhh0"][k][H:2 * H]),
            0.5 * (r["bih1"][k][:H] + r["bhh1"][k][:H]),
            0.5 * (r["bih1"][k][H:2 * H] + r["bhh1"][k][H:2 * H])], 1)
        m = dict(common)
        m.update({
            "xT": xT.astype(f32),
            "Wih0T": np.ascontiguousarray(r["Wih0"][k].T).astype(f32),
            "Whh0T": np.ascontiguousarray(
                (r["Whh0"][k] * np.concatenate(
                    [np.ones(2 * H), 0.5 * np.ones(H)])[:, None]).T).astype(f32),
            "Wih1T": np.ascontiguousarray((0.5 * r["Wih1"][k]).T).astype(f32),
            "Whh1T": np.ascontiguousarray(
                (r["Whh1"][k] * np.concatenate(
                    [np.ones(2 * H), 0.5 * np.ones(H)])[:, None]).T).astype(f32),
            "brz": brz_cols.astype(f32),
            "bias2_0": np.stack([r["bih0"][k][:H] + r["bhh0"][k][:H],
                                 r["bih0"][k][H:2 * H] + r["bhh0"][k][H:2 * H]]).astype(f32),
            "bias2_1": np.stack([r["bih1"][k][:H] + r["bhh1"][k][:H],
                                 r["bih1"][k][H:2 * H] + r["bhh1"][k][H:2 * H]]).astype(f32),
            "bin01": np.stack([r["bih0"][k][2 * H:], r["bih1"][k][2 * H:]], 1).astype(f32),
            "bhn_rows": (0.5 * np.concatenate(
                [r["bhh0"][k][2 * H:], r["bhh1"][k][2 * H:]])).reshape(1, 256).astype(f32),
            "WoutX": (0.5 * Wout[0, k * H:(k + 1) * H]).reshape(128, 1).astype(bf),
        })
        maps.append(m)
    return maps


_CACHE = {}


def kernel(**inputs) -> np.ndarray:
    nsteps = NSTEPS
    if "nc" not in _CACHE:
        _CACHE["nc"] = build(nsteps)
    nc = _CACHE["nc"]
    maps = make_inmaps(inputs, nsteps)
    res = run_bass_kernel_spmd(nc, maps, core_ids=list(range(8)))
    outs = res.results
    L = nsteps
    B = 128
    p1 = np.zeros((B, L), np.float32)
    p2 = np.zeros((B, L), np.float32)
    for core in range(8):
        k, shard = core // 4, core % 4
        bsl = slice(shard * BC, (shard + 1) * BC)
        p = np.asarray(outs[core]["p_out"]).reshape(L, BC).T   # [32, L]
        (p1 if k == 0 else p2)[bsl] = p
    bout = float(np.asarray(inputs["bout"]).reshape(-1)[0])
    idx = np.minimum(np.arange(L) + 1, L - 1)
    z = p1 + p2[:, idx] + bout
    out = (1.0 / (1.0 + np.exp(-z))).astype(np.float32)[..., None]
    return out
